# revision 18
# baseline (speedup 1.0000x reference)
"""Trainium2 Bass kernel for CTC beam-search decoding (nn_CTCPredictionsCpu).

Contract: kernel(data [128,64,32] f32, data_length [64] int32) -> preds [64,128] int32.

Strategy (pure data parallelism): 64 samples -> 8 cores x 8 samples.
Per core: 8 samples x 16 beams = 128 SBUF partitions, p = s*16 + i.

Device algorithm (per core):
  - log_softmax over classes, then "poison" frames t >= len(s):
    logp[blank]=0, others=NEG. Poisoned steps preserve lp_tot/prefixes,
    so no per-step length masking is needed.
  - CTC prefix beam search with per-beam state in partition p:
    lp_b, lp_nb, lp_tot, len, last, two rolling hashes (mod 8192) of the
    prefix, and the prefix chars themselves [128 cols].
  - prefix-merge detection: beam j extends beam i with char c=last_j iff
    hX_j - last_j - 1 - hX_i*AX = 0 (mod 8192)  for X in {A,B}
    and len_j == len_i + 1. The j-side quantities (XA=hA-last-1, XB,
    last, lenm, eg) are broadcast to all rows of a sample via one
    block-diagonal PE matmul; the i-side (npk3X=-hX*AX, lenp1=len+1)
    are per-partition scalars.
  - top-16-of-(16x33) selection: per-beam top-16 via max8/match_replace,
    then per-sample global top-16 on a PE-broadcast 256-vec; source beam
    decoded positionally via max_index, chosen char by value match.
  - state update via a one-hot selection matrix SEL and PE gather matmuls.

Engine split: serial spine on DVE/PE/Act; independent scalar prep on
Pool (gpsimd) so its sequencer absorbs the small ops.
"""
import os
import numpy as np

T, B, C = 128, 64, 32
BEAM = 16
BLANK = C - 1
NEG = np.float32(-1e30)
A1, A2 = 1031.0, 1537.0
NCORES = 8
SPC = B // NCORES          # samples per core = 8
P = 128                    # partitions
L = T                      # max prefix length

_cached = {}


def _build_nc(nsteps=T, debug=False):
    import concourse.bass as bass
    import concourse.bacc as bacc
    import concourse.mybir as mybir
    from concourse import tile
    from contextlib import ExitStack

    f32 = mybir.dt.float32
    i32 = mybir.dt.int32
    u32 = mybir.dt.uint32
    Alu = mybir.AluOpType
    Act = mybir.ActivationFunctionType

    nc = bacc.Bacc("TRN2", target_bir_lowering=False, debug=False,
                   num_devices=NCORES)

    # ---- DRAM I/O ----
    d_data = nc.dram_tensor("data", [P, T, C], f32, kind="ExternalInput")
    d_lens = nc.dram_tensor("lens", [P, 1], f32, kind="ExternalInput")
    d_state0 = nc.dram_tensor("state0", [P, 16], f32, kind="ExternalInput")
    d_gv0 = nc.dram_tensor("gv0", [P, 16], f32, kind="ExternalInput")
    d_consts = nc.dram_tensor("consts", [P, 512], f32, kind="ExternalInput")
    d_out = nc.dram_tensor("preds", [SPC, T], i32, kind="ExternalOutput")
    d_dbg = {}
    if debug:
        for nm, sh in [("dstate", [P, 16]), ("dgv", [P, 16]),
                       ("dpref", [P, L]), ("dcg", [P, 36]),
                       ("dpsg", [P, 167]), ("dval", [P, 1]),
                       ("didx", [P, 16]), ("dexps", [P, 4])]:
            d_dbg[nm] = nc.dram_tensor(nm, sh, f32, kind="ExternalOutput")

    # consts layout (f32 cols):
    #  0:32   IOTA32 (0..31)
    #  65:193 IOTA_L (0..127)
    #  193:321 BLKDIAG row (q-th row: 1.0 at cols of q's sample block)
    #  321:337 BEAMMASK (1 at col p%16)
    #  337:345 BLK8 (1 at col p//16)
    #  346:347 ONESCOL = 1.0
    #  355:356 BEAMIDX = p%16
    #  356:364 INM8INIT = -3e38
    #  364:366 A1A2T = [A1, A2]
    #  366:368 NA1A2T = [-A1, -A2]

    with tile.TileContext(nc) as tc, ExitStack() as ctx:
        pool = ctx.enter_context(tc.tile_pool(name="main", bufs=1))
        tpool = ctx.enter_context(tc.tile_pool(name="tmp", bufs=2))
        psum = ctx.enter_context(tc.tile_pool(name="ps", bufs=1, space="PSUM"))
        psum2 = ctx.enter_context(tc.tile_pool(name="ps2", bufs=1, space="PSUM"))

        # ---- persistent tiles ----
        LOGP = pool.tile([P, T, C], f32)     # poisoned; col31=NEG after prep
        LOGPB = pool.tile([P, T], f32)       # blank logp per t (0 if t>=len)
        CONS = pool.tile([P, 512], f32)
        STATE = pool.tile([P, 12], f32)
        # STATE cols: hA(0) hB(1) last(2) lp_b(3) lp_nb(4) lenm(5) lp_tot(6)
        #             npk3A(7) npk3B(8) nlast(9) nlenp1(10)
        GV = pool.tile([P, 16], f32)         # global top16 (col0 = Ms)
        PREF = pool.tile([P, L], f32)
        LENSM = pool.tile([P, 1], f32)
        CGX = pool.tile([P, 36], f32)
        # CGX cols: stay_b(0) stay_nb(1) lenm(2) stay_score(3) ext2(4:36)
        INM8 = pool.tile([P, 8], f32)

        IOTA32 = CONS[:, 0:32]
        IOTA_L = CONS[:, 65:193]
        BLKDIAG = CONS[:, 193:321]
        BEAMMASK = CONS[:, 321:337]
        BLK8 = CONS[:, 337:345]
        ONESCOL = CONS[:, 346:347]
        BEAMIDX = CONS[:, 355:356]
        INM8INIT = CONS[:, 356:364]
        A1A2T = CONS[:, 364:366]
        NA1A2T = CONS[:, 366:368]

        # ---- load inputs ----
        STATE_raw = pool.tile([P, 16], f32)
        GV_raw = pool.tile([P, 16], f32)
        CONS_raw = pool.tile([P, 512], f32)
        LENS_raw = pool.tile([P, 1], f32)
        nc.sync.dma_start(LOGP[:], d_data[:])
        nc.sync.dma_start(CONS_raw[:], d_consts[:])
        nc.sync.dma_start(STATE_raw[:], d_state0[:])
        nc.sync.dma_start(GV_raw[:], d_gv0[:])
        nc.sync.dma_start(LENS_raw[:], d_lens[:])
        nc.vector.tensor_copy(STATE[:], STATE_raw[:, 0:12])
        nc.vector.tensor_copy(GV[:], GV_raw[:])
        nc.vector.tensor_copy(CONS[:], CONS_raw[:])
        nc.vector.tensor_copy(LENSM[:], LENS_raw[:])
        nc.vector.tensor_copy(INM8[:], INM8INIT[:])
        nc.vector.memset(PREF[:], 0.0)
        nc.vector.memset(CGX[:], float(NEG))

        # ---- log_softmax over c for each (p, t); then poison ----
        MX = pool.tile([P, T], f32)
        SM = pool.tile([P, T], f32)
        lv = LOGP[:]
        nc.vector.tensor_reduce(MX[:], lv, axis=mybir.AxisListType.X,
                                op=Alu.max)
        mxb = MX[:].unsqueeze(2).broadcast_to([P, T, C])
        nc.vector.tensor_tensor(lv, lv, mxb, op=Alu.subtract)
        EXPV = pool.tile([P, T, C], f32)
        nc.scalar.activation(EXPV[:], lv, Act.Exp)
        nc.vector.tensor_reduce(SM[:], EXPV[:], axis=mybir.AxisListType.X,
                                op=Alu.add)
        nc.scalar.activation(SM[:], SM[:], Act.Ln)
        smb = SM[:].unsqueeze(2).broadcast_to([P, T, C])
        nc.vector.tensor_tensor(lv, lv, smb, op=Alu.subtract)

        # poison: for t >= len: all cols NEG, and LOGPB (blank col) = 0
        AM = pool.tile([P, T], f32)   # active mask: t < len
        nc.vector.tensor_scalar(AM[:], IOTA_L[:, 0:T], LENSM[:, 0:1], None,
                                op0=Alu.is_lt)
        l31 = LOGP[:][:, :, BLANK:BLANK+1]
        nc.vector.tensor_tensor(LOGPB[:], l31.squeeze(2), AM[:], op=Alu.mult)
        IAM = pool.tile([P, T], f32)
        nc.vector.tensor_scalar(IAM[:], AM[:], -float(NEG), float(NEG),
                                op0=Alu.mult, op1=Alu.add)  # NEG*(1-active)
        amb = AM[:].unsqueeze(2).broadcast_to([P, T, C])
        nc.vector.tensor_tensor(lv, lv, amb, op=Alu.mult)
        iamb = IAM[:].unsqueeze(2).broadcast_to([P, T, C])
        nc.vector.tensor_tensor(lv, lv, iamb, op=Alu.add)
        nc.vector.memset(l31.squeeze(2), float(NEG))

        tp = tpool
        lastp = STATE[:, 2:3]
        lptotp = STATE[:, 6:7]

        def step(t_idx, dbg=False):
            LT = LOGP[:][:, t_idx, :]
            LBT = LOGPB[:, t_idx:t_idx+1]

            # ---- early: depends only on step-start STATE/GV/LOGP ----
            OH = tp.tile([P, C], f32, tag="OH")
            nc.vector.tensor_scalar(OH[:], IOTA32, lastp, None,
                                    op0=Alu.is_equal)
            JKS = tp.tile([P, C], f32, tag="JKS")
            GT = tp.tile([P, 1], f32, tag="GT")
            nc.vector.tensor_tensor(JKS[:], OH[:], LT, op=Alu.mult)
            nc.vector.tensor_reduce(GT[:], JKS[:], axis=mybir.AxisListType.X,
                                    op=Alu.add)
            RB = tp.tile([P, 4], f32, tag="RB")
            # RB: XA(0) XB(1) last(2) lenm(3)
            nc.vector.tensor_scalar(RB[:, 0:2], STATE[:, 0:2], lastp, -1.0,
                                    op0=Alu.subtract, op1=Alu.add)
            nc.vector.tensor_copy(RB[:, 2:4], STATE[:, 2:6:3])  # last, lenm
            EGT = tp.tile([P, 1], f32, tag="EGT")
            nc.scalar.activation(EGT[:], GT[:], Act.Exp)   # eg
            NPK4 = tp.tile([P, 64], f32, tag="NPK4")
            nc.gpsimd.tensor_copy(
                NPK4[:].rearrange("p (a b) -> p a b", a=4),
                STATE[:, 7:11].unsqueeze(2).broadcast_to([P, 4, 16]))

            NEGMS = tp.tile([P, 1], f32, tag="NEGMS")
            nc.gpsimd.tensor_scalar(NEGMS[:], GV[:, 0:1], -1.0, None,
                                    op0=Alu.mult)
            A4 = tp.tile([P, 4], f32, tag="A4")
            # A4: lp_b(0) lp_tot(1) lp_nb+G(2) stay_b(3)
            nc.gpsimd.tensor_copy(A4[:, 0:1], STATE[:, 3:4])
            nc.gpsimd.tensor_copy(A4[:, 1:2], STATE[:, 6:7])
            nc.gpsimd.tensor_tensor(A4[:, 2:3], STATE[:, 4:5], GT[:],
                                    op=Alu.add)
            nc.gpsimd.tensor_tensor(A4[:, 3:4], STATE[:, 6:7], LBT,
                                    op=Alu.add)
            nc.gpsimd.tensor_copy(CGX[:, 0:1], A4[:, 3:4])     # stay_b
            nc.gpsimd.tensor_copy(CGX[:, 2:3], STATE[:, 5:6])  # lenm
            EXPS = tp.tile([P, 4], f32, tag="EXPS")
            nc.scalar.activation(EXPS[:], A4[:], Act.Exp, bias=NEGMS[:, 0:1])
            # EXPS: elpb(0) elpt(1) e_nb_base(2) e_b(3)
            DD = tp.tile([P, 1], f32, tag="DD")
            nc.gpsimd.tensor_tensor(DD[:], STATE[:, 3:4], STATE[:, 6:7],
                                    op=Alu.subtract)
            DE = tp.tile([P, 1], f32, tag="DE")
            nc.gpsimd.tensor_tensor(DE[:], EXPS[:, 0:1], EXPS[:, 1:2],
                                    op=Alu.subtract)
            EXTT = tp.tile([P, C], f32, tag="EXTT")
            nc.gpsimd.tensor_tensor(EXTT[:], LT,
                                    lptotp.broadcast_to([P, C]), op=Alu.add)
            EXTD = tp.tile([P, C], f32, tag="EXTD")
            nc.gpsimd.tensor_tensor(EXTD[:], OH[:],
                                    DD[:, 0:1].broadcast_to([P, C]),
                                    op=Alu.mult)
            EXTT2 = tp.tile([P, C], f32, tag="EXTT2")
            nc.gpsimd.tensor_tensor(EXTT2[:], EXTD[:], EXTT[:], op=Alu.add)

            # ---- broadcast j-side state to all rows of the sample ----
            RHSA = tp.tile([P, 64], f32, tag="RHSA")
            rb4 = RB[:].unsqueeze(2).broadcast_to([P, 4, 16])
            bm4 = BEAMMASK.unsqueeze(1).broadcast_to([P, 4, 16])
            nc.vector.tensor_tensor(
                RHSA[:].rearrange("p (a b) -> p a b", a=4), rb4, bm4,
                op=Alu.mult)
            psBC = psum.tile([P, 64], f32, tag="psBC")
            nc.tensor.matmul(psBC[:], BLKDIAG, RHSA[:], start=True, stop=True)
            # psBC: XA16(0:16) XB16(16:32) last16(32:48) lenm16(48:64)
            RHSE = tp.tile([P, 16], f32, tag="RHSE")
            nc.vector.tensor_scalar(RHSE[:], BEAMMASK, EGT[:, 0:1], None,
                                    op0=Alu.mult)
            psBCE = psum.tile([P, 16], f32, tag="psBCE")
            nc.tensor.matmul(psBCE[:], BLKDIAG, RHSE[:], start=True,
                             stop=True)  # eg16

            # ---- hash/last/len match: one 64-wide test ----
            # cols 0:16 hashA, 16:32 hashB, 32:48 last-eq, 48:64 len-eq;
            # the differences are all in (-8192, 8192), so the &8191 mod
            # test is an exact equality test for every column group.
            VIF = tp.tile([P, 64], f32, tag="VIF")
            nc.vector.tensor_tensor(VIF[:], psBC[:], NPK4[:], op=Alu.add)
            VI = tp.tile([P, 64], i32, tag="VI")
            nc.vector.tensor_copy(VI[:], VIF[:])
            nc.vector.tensor_scalar(VI[:], VI[:], 8191, None,
                                    op0=Alu.bitwise_and)
            E4F = tp.tile([P, 64], f32, tag="E4F")
            nc.vector.tensor_scalar(E4F[:], VI[:], 0, None, op0=Alu.is_equal)
            MTAB = tp.tile([P, 16], f32, tag="MTAB")
            nc.vector.tensor_tensor(MTAB[:], E4F[:, 0:16], E4F[:, 16:32],
                                    op=Alu.mult)
            MT = tp.tile([P, 16], f32, tag="MT")
            nc.vector.tensor_tensor(MT[:], MTAB[:], E4F[:, 48:64],
                                    op=Alu.mult)

            # ---- merged mass ----
            Q16 = tp.tile([P, 16], f32, tag="Q16")
            nc.vector.scalar_tensor_tensor(
                Q16[:], E4F[:, 32:48], DE[:, 0:1],
                EXPS[:, 1:2].broadcast_to([P, 16]),
                op0=Alu.mult, op1=Alu.add)
            CONTRIB = tp.tile([P, 16], f32, tag="CONTRIB")
            nc.vector.tensor_tensor(CONTRIB[:], MT[:], Q16[:], op=Alu.mult)
            CONTRIB2 = tp.tile([P, 16], f32, tag="CONTRIB2")
            nc.vector.tensor_tensor(CONTRIB2[:], CONTRIB[:], psBCE[:],
                                    op=Alu.mult)

            MM = tp.tile([P, 128], f32, tag="MM")
            mt8 = MT[:].unsqueeze(1).broadcast_to([P, 8, 16])
            blk8b = BLK8.unsqueeze(2).broadcast_to([P, 8, 16])
            nc.vector.tensor_tensor(
                MM[:].rearrange("p (a b) -> p a b", a=8), mt8, blk8b,
                op=Alu.mult)
            MMT = tp.tile([P, 128], f32, tag="MMT")
            nc.vector.transpose(MMT[:], MM[:])
            psMRG = psum.tile([P, C], f32, tag="psMRG")
            nc.tensor.matmul(psMRG[:], MMT[:], OH[:], start=True, stop=True)

            CM = tp.tile([P, 128], f32, tag="CM")
            cb8 = CONTRIB2[:].unsqueeze(1).broadcast_to([P, 8, 16])
            nc.vector.tensor_tensor(
                CM[:].rearrange("p (a b) -> p a b", a=8), cb8, blk8b,
                op=Alu.mult)
            psS = psum.tile([P, 1], f32, tag="psS")
            nc.tensor.matmul(psS[:], CM[:], ONESCOL, start=True, stop=True)

            SNL = tp.tile([P, 2], f32, tag="SNL")
            nc.vector.tensor_tensor(SNL[:, 0:1], EXPS[:, 2:3], psS[:],
                                    op=Alu.add)
            nc.vector.tensor_tensor(SNL[:, 1:2], SNL[:, 0:1], EXPS[:, 3:4],
                                    op=Alu.add)
            LNS = tp.tile([P, 2], f32, tag="LNS")
            nc.scalar.activation(LNS[:], SNL[:], Act.Ln)
            # stay_nb -> CGX[1], stay_score -> CGX[3] (strided write)
            nc.vector.tensor_scalar(CGX[:, 1:4:2], LNS[:], GV[:, 0:1], -1e38,
                                    op0=Alu.add, op1=Alu.max)
            nc.vector.scalar_tensor_tensor(
                CGX[:, 4:36], psMRG[:], -2e30, EXTT2[:],
                op0=Alu.mult, op1=Alu.add)

            # ---- selection ----
            V16 = tp.tile([P, 16], f32, tag="V16")
            CX = tp.tile([P, 33], f32, tag="CX")
            nc.vector.max(V16[:, 0:8], CGX[:, 3:36])
            nc.vector.match_replace(CX[:], V16[:, 0:8], CGX[:, 3:36], -3e38)
            nc.vector.max(V16[:, 8:16], CX[:])

            RHSB = tp.tile([P, 256], f32, tag="RHSB")
            v16b = V16[:].unsqueeze(2).broadcast_to([P, 16, 16])
            bm16 = BEAMMASK.unsqueeze(1).broadcast_to([P, 16, 16])
            nc.vector.tensor_tensor(
                RHSB[:].rearrange("p (a b) -> p a b", a=16), v16b, bm16,
                op=Alu.mult)
            psBV = psum2.tile([P, 256], f32, tag="psBV")
            nc.tensor.matmul(psBV[:], BLKDIAG, RHSB[:], start=True, stop=True)
            BCV2 = tp.tile([P, 256], f32, tag="BCV2")
            nc.vector.max(GV[:, 0:8], psBV[:])
            nc.vector.match_replace(BCV2[:], GV[:, 0:8], psBV[:], -3e38)
            nc.vector.max(GV[:, 8:16], BCV2[:])

            JKG = tp.tile([P, 16], f32, tag="JKG")
            nc.vector.tensor_tensor(JKG[:], GV[:], BEAMMASK, op=Alu.mult)
            nc.vector.tensor_reduce(STATE[:, 6:7], JKG[:],
                                    axis=mybir.AxisListType.X, op=Alu.add)
            VALT = STATE[:, 6:7]    # lp_tot' = selected score
            # FOUND[p, b] = 1 iff global candidate b's value is in row p's
            # own per-beam top-16 list (values distinct across beams).
            FEQ = tp.tile([P, 256], f32, tag="FEQ")
            gvb = GV[:].unsqueeze(2).broadcast_to([P, 16, 16])
            v16u = V16[:].unsqueeze(1).broadcast_to([P, 16, 16])
            nc.vector.tensor_tensor(
                FEQ[:].rearrange("p (a b) -> p a b", a=16), gvb, v16u,
                op=Alu.is_equal)
            FOUND = tp.tile([P, 16], f32, tag="FOUND")
            nc.vector.tensor_reduce(
                FOUND[:].unsqueeze(2),
                FEQ[:].rearrange("p (a b) -> p a b", a=16),
                axis=mybir.AxisListType.X, op=Alu.max)
            SEL = tp.tile([P, 128], f32, tag="SEL")
            fnd8 = FOUND[:].unsqueeze(1).broadcast_to([P, 8, 16])
            nc.vector.tensor_tensor(
                SEL[:].rearrange("p (a b) -> p a b", a=8), fnd8, blk8b,
                op=Alu.mult)

            # ---- gather ----
            psG = psum2.tile([P, 167], f32, tag="psG")
            nc.tensor.matmul(psG[:, 0:3], SEL[:], STATE[:, 0:3],
                             start=True, stop=True)
            nc.tensor.matmul(psG[:, 3:39], SEL[:], CGX[:],
                             start=True, stop=True)
            nc.tensor.matmul(psG[:, 39:167], SEL[:], PREF[:],
                             start=True, stop=True)
            # psG: hA(0) hB(1) last(2) stay_b(3) stay_nb(4) lenm(5)
            #      stay_score(6) ext(7:39) pref(39:167)

            # ---- decode chosen char by value match in gathered ext row ----
            nc.vector.tensor_copy(INM8[:, 0:1], VALT)
            IDX8 = tp.tile([P, 8], u32, tag="IDX8")
            nc.vector.max_index(IDX8[:], INM8[:], psG[:, 7:39])
            IDXF1 = tp.tile([P, 1], f32, tag="IDXF1")
            nc.vector.tensor_copy(IDXF1[:], IDX8[:, 0:1])
            CODEF = tp.tile([P, 1], f32, tag="CODEF")
            nc.vector.tensor_scalar(CODEF[:], IDXF1[:], 1.0, None,
                                    op0=Alu.add)
            ISST = tp.tile([P, 1], i32, tag="ISST")
            nc.vector.tensor_scalar(ISST[:], IDXF1[:], 1e9, None,
                                    op0=Alu.is_gt)
            ISEX = tp.tile([P, 1], f32, tag="ISEX")
            nc.vector.tensor_scalar(ISEX[:], IDXF1[:], 1e9, None,
                                    op0=Alu.is_lt)

            # ---- writeback: ext-case values, then stay-case overwrite ----
            TT0 = tp.tile([P, 2], f32, tag="TT0")
            nc.vector.tensor_tensor(TT0[:], psG[:, 0:2], A1A2T, op=Alu.mult)
            TT0I = tp.tile([P, 2], i32, tag="TT0I")
            nc.vector.tensor_scalar(TT0I[:], TT0[:], CODEF[:, 0:1], None,
                                    op0=Alu.add)
            nc.vector.tensor_scalar(TT0I[:], TT0I[:], 8191, None,
                                    op0=Alu.bitwise_and)
            nc.vector.tensor_copy(STATE[:, 0:2], TT0I[:])       # hA' hB'
            nc.vector.tensor_scalar(STATE[:, 2:3], CODEF[:], -1.0, None,
                                    op0=Alu.add)                # last' = c
            nc.vector.memset(STATE[:, 3:4], float(NEG))         # lp_b'
            nc.vector.tensor_copy(STATE[:, 4:5], VALT)          # lp_nb'
            nc.vector.copy_predicated(STATE[:, 0:5],
                                      ISST[:, 0:1].broadcast_to([P, 5]),
                                      psG[:, 0:5])
            nc.vector.tensor_tensor(STATE[:, 5:6], psG[:, 5:6], ISEX[:],
                                    op=Alu.add)                 # lenm'
            nc.vector.tensor_tensor(STATE[:, 7:9], STATE[:, 0:2], NA1A2T,
                                    op=Alu.mult)                # npk3'
            nc.vector.tensor_scalar(STATE[:, 9:10], STATE[:, 2:3], -1.0,
                                    None, op0=Alu.mult)         # nlast'
            nc.vector.tensor_scalar(STATE[:, 10:11], STATE[:, 5:6], -1.0,
                                    -1.0, op0=Alu.mult, op1=Alu.add)
            # nlenp1' = -(lenm'+1)
            # ---- prefix update ----
            MSK = tp.tile([P, L], i32, tag="MSK")
            nc.vector.scalar_tensor_tensor(
                MSK[:], IOTA_L, psG[:, 5:6],
                ISEX[:, 0:1].broadcast_to([P, L]),
                op0=Alu.is_equal, op1=Alu.mult)
            nc.scalar.copy(PREF[:], psG[:, 39:167])
            nc.vector.copy_predicated(PREF[:], MSK[:],
                                      STATE[:, 2:3].broadcast_to([P, L]))
            if dbg:
                nc.sync.dma_start(d_dbg["dcg"][:], CGX[:])
                DPS = tp.tile([P, 167], f32, tag="DPS")
                nc.vector.tensor_copy(DPS[:], psG[:])
                nc.sync.dma_start(d_dbg["dpsg"][:], DPS[:])
                nc.sync.dma_start(d_dbg["dval"][:], VALT[:])
                nc.sync.dma_start(d_dbg["didx"][:], J16[:])
                nc.sync.dma_start(d_dbg["dexps"][:], EXPS[:])

        for t_idx in range(nsteps):
            step(t_idx, dbg=(debug and t_idx == nsteps - 1))
        if debug:
            DST = pool.tile([P, 16], f32)
            nc.vector.memset(DST[:], 0.0)
            nc.vector.tensor_copy(DST[:, 0:12], STATE[:])
            nc.sync.dma_start(d_dbg["dstate"][:], DST[:])
            nc.sync.dma_start(d_dbg["dgv"][:], GV[:])
            nc.sync.dma_start(d_dbg["dpref"][:], PREF[:])

        # ---- output ----
        PM = pool.tile([P, L], f32)
        nc.vector.tensor_scalar(PM[:], IOTA_L, LENSM[:, 0:1], None,
                                op0=Alu.is_lt)
        nc.vector.tensor_tensor(PM[:], PM[:], PREF[:], op=Alu.mult)
        OUTI = pool.tile([P, L], i32)
        nc.vector.tensor_copy(OUTI[:], PM[:])
        nc.sync.dma_start(
            d_out[:], OUTI[:].rearrange("(s b) l -> s b l", b=16)[:, 0:1, :])

    nc.compile()
    _dedupe_act_table_loads(nc, mybir)
    return nc


def _dedupe_act_table_loads(nc, mybir):
    """All activations here are Exp/Ln/Copy/Identity — table set 6
    (natural_log_exp_and_others) serves every one. The placement pass
    alternates exp_and_others(0) / natural_log(5), costing a 1283ns
    table load twice per step. Keep the first load, retarget it to the
    combined set, drop the rest (they carry no sync_info)."""
    first = True
    for b in nc.main_func.blocks:
        keep = []
        for i in b.instructions:
            if isinstance(i, mybir.InstLoadActFuncSet):
                assert i.sync_info is None or (
                    not i.sync_info.on_wait and not i.sync_info.on_update)
                if first:
                    i.act_func_set_id = 6
                    first = False
                    keep.append(i)
                continue
            keep.append(i)
        b.instructions = keep


def _host_consts():
    cons = np.zeros((P, 512), np.float32)
    cons[:, 0:32] = np.arange(32, dtype=np.float32)[None, :]
    cons[:, 65:193] = np.arange(L, dtype=np.float32)[None, :]
    blkdiag = np.zeros((P, 128), np.float32)
    for s in range(8):
        blkdiag[s*16:(s+1)*16, s*16:(s+1)*16] = 1.0
    cons[:, 193:321] = blkdiag
    beammask = np.zeros((P, 16), np.float32)
    beammask[np.arange(P), np.arange(P) % 16] = 1.0
    cons[:, 321:337] = beammask
    blk8 = np.zeros((P, 8), np.float32)
    blk8[np.arange(P), np.arange(P) // 16] = 1.0
    cons[:, 337:345] = blk8
    cons[:, 346:347] = 1.0
    cons[:, 355:356] = (np.arange(P) % 16).astype(np.float32)[:, None]
    cons[:, 356:364] = -3e38
    cons[:, 364] = A1
    cons[:, 365] = A2
    cons[:, 366] = -A1
    cons[:, 367] = -A2
    return cons


def _host_state0():
    st = np.zeros((P, 16), np.float32)
    beam = np.arange(P) % 16
    live = beam == 0
    st[:, 0] = 0.0    # hA
    st[:, 1] = 0.0    # hB
    st[:, 2] = -1.0   # last
    st[:, 3] = np.where(live, 0.0, NEG)       # lp_b
    st[:, 4] = NEG                            # lp_nb
    st[:, 5] = np.where(live, 0.0, -1000.0)   # lenm
    st[:, 6] = np.where(live, 0.0, NEG)       # lp_tot
    st[:, 7] = 0.0    # npk3A = -hA*A1
    st[:, 8] = 0.0    # npk3B
    st[:, 9] = 1.0    # nlast = -last
    st[:, 10] = -(st[:, 5] + 1.0)             # nlenp1 = -(lenm+1)
    gv0 = np.full((P, 16), NEG, np.float32)
    gv0[:, 0] = 0.0   # Ms at step 0
    return st, gv0


def kernel(data, data_length):
    import sys
    if "/opt/trn_rl_repo" not in sys.path:
        sys.path.insert(0, "/opt/trn_rl_repo")
    from concourse.bass_utils import run_bass_kernel_spmd

    data = np.asarray(data, np.float32)
    lens = np.asarray(data_length)

    if "nc" not in _cached:
        _cached["nc"] = _build_nc()
        _cached["consts"] = _host_consts()
        _cached["state0"] = _host_state0()
    nc = _cached["nc"]
    cons = _cached["consts"]
    st0, gv0 = _cached["state0"]

    in_maps = []
    for core in range(NCORES):
        s0 = core * SPC
        d = np.repeat(data[:, s0:s0+SPC, :], BEAM, axis=1)
        d = np.ascontiguousarray(d.transpose(1, 0, 2))
        lc = np.repeat(lens[s0:s0+SPC].astype(np.float32), BEAM)[:, None]
        in_maps.append({
            "data": d, "lens": np.ascontiguousarray(lc),
            "state0": st0, "gv0": gv0, "consts": cons,
        })

    _cached["last_in_maps"] = in_maps
    res = run_bass_kernel_spmd(nc, in_maps, list(range(NCORES)))
    out = np.zeros((B, T), np.int32)
    for core in range(NCORES):
        out[core*SPC:(core+1)*SPC] = res.results[core]["preds"]
    return out
